# revision 7
# baseline (speedup 1.0000x reference)
"""AdditiveAttention Trainium2 kernel.

reference:
    q_proj  = query @ Wq.T                              (B,H)
    k_proj  = einsum('bsh,gh->bsg', keys, Wk)           (B,S,H)
    scores  = einsum('bsh,h->bs', tanh(q_proj[:,None,:]+k_proj), v)
    scores  = where(mask, -1e9, scores)
    weights = softmax(scores, axis=1)                   (B,S)
    context = einsum('bs,bsh->bh', weights, keys)       (B,H)
    return (context, weights)

Sharding: data-parallel over batch, 4 batches per core on 8 cores.
Wk/Wq/v replicated. Host does layout-only transforms (transposes /
reshapes); all FLOPs run on device.

Per-core device pipeline (B'=4 local batches):
  - keysT (h on partitions) resident in SBUF: (128, 16, 2048) f32.
  - k_projT tile (g=128, s=512) = sum_ht WkT[ht,g].T @ keysT[ht,s]
    as fp32r matmuls (full PE rate at N=512), accumulated in PSUM.
  - tanh fused with q_proj bias on ScalarE: tanh(kp + qp[g]) -> SBUF.
  - scores chunk (1,512) = sum_gt v[gt].T @ combined[gt]  (M=1 matmuls).
  - mask (copy_predicated to -88) + Exp (accum_out = chunk sum) on chunk.
  - u chunk -> DRAM bounce -> DMA partition-broadcast (128,512) ->
    context via fused tensor_tensor_reduce on VectorE against keysT
    (contraction over s without needing a second keys layout).
  - weights = u * 1/sum  (per-batch sums), context scaled likewise.
"""

import sys

for _p in ("/opt/trn_rl_repo",):
    if _p not in sys.path:
        sys.path.insert(0, _p)

import numpy as np

import concourse.bass as bass
import concourse.bacc as bacc
import concourse.tile as tile
from concourse import mybir
from concourse import bass_utils

F32 = mybir.dt.float32
F32R = mybir.dt.float32r
U8 = mybir.dt.uint8
AF = mybir.ActivationFunctionType
ALU = mybir.AluOpType
AX = mybir.AxisListType

B, S, H = 32, 2048, 512
N_CORES = 8
BPC = B // N_CORES  # batches per core

NEG_MASK = -88.0  # exp(-88) == 0 in fp32; avoids feeding -1e9 to the Exp LUT


def build_program(bpc=BPC, s=S, h=H, kproj_dtype=F32R):
    """Build + compile the per-core Bass program (SPMD, same on all cores)."""
    nc = bacc.Bacc("TRN2", target_bir_lowering=False, debug=False)
    ht_n = h // 128   # tiles along contraction h
    gt_n = h // 128   # tiles along output g
    sc_n = s // 512   # 512-wide score chunks
    sc_w = 512

    kd = kproj_dtype
    keysT_d = nc.dram_tensor("keysT", (bpc, h, s), kd, kind="ExternalInput")
    qT_d = nc.dram_tensor("queryT", (h, bpc), F32, kind="ExternalInput")
    mask_d = nc.dram_tensor("mask_u8", (1, bpc * s), U8, kind="ExternalInput")
    wkT_d = nc.dram_tensor("wkT", (h, h), kd, kind="ExternalInput")
    wqT_d = nc.dram_tensor("wqT", (h, h), F32, kind="ExternalInput")
    v_d = nc.dram_tensor("v4", (128, gt_n), kd, kind="ExternalInput")
    ctx_d = nc.dram_tensor("ctx", (bpc, h), F32, kind="ExternalOutput")
    w_d = nc.dram_tensor("w", (bpc, s), F32, kind="ExternalOutput")

    with tile.TileContext(nc) as tc:
        with (
            tc.tile_pool(name="consts", bufs=1) as consts,
            tc.tile_pool(name="keysp", bufs=1) as keysp,
            tc.tile_pool(name="work", bufs=3) as work,
            tc.tile_pool(name="combp", bufs=4) as combp,
            tc.tile_pool(name="psum", bufs=3, space="PSUM") as psum,
            tc.tile_pool(name="dram", bufs=1, space="DRAM") as dram,
        ):
            # ---- resident inputs ----
            keys_sb = keysp.tile([128, bpc * ht_n, s], kd)
            keysT_r = keysT_d.ap().rearrange("b (ht p) s -> p (b ht) s", p=128)
            for b in range(bpc):
                for ht in range(ht_n):
                    i = b * ht_n + ht
                    nc.sync.dma_start(keys_sb[:, i, :], keysT_r[:, i, :])

            wk_sb = consts.tile([128, ht_n, h], kd)
            nc.sync.dma_start(wk_sb[:], wkT_d.ap().rearrange("(ht p) g -> p ht g", p=128))
            wq_sb = consts.tile([128, ht_n, h], F32)
            nc.sync.dma_start(wq_sb[:], wqT_d.ap().rearrange("(ht p) g -> p ht g", p=128))
            qT_sb = consts.tile([128, ht_n, bpc], F32)
            nc.sync.dma_start(qT_sb[:], qT_d.ap().rearrange("(ht p) c -> p ht c", p=128))
            v_sb = consts.tile([128, gt_n], kd)
            nc.sync.dma_start(v_sb[:], v_d.ap())
            mask_sb = consts.tile([1, bpc * s], U8)
            nc.sync.dma_start(mask_sb[:], mask_d.ap())
            neg_sb = consts.tile([1, sc_w], F32)
            nc.vector.memset(neg_sb[:], NEG_MASK)

            # ---- q_proj (exact fp32; tiny) ----
            qp_sb = consts.tile([128, gt_n * bpc], F32)
            for gt in range(gt_n):
                qp_ps = psum.tile([128, bpc], F32, tag="qp", bufs=1)
                for ht in range(ht_n):
                    nc.tensor.matmul(
                        qp_ps[:],
                        wq_sb[:, ht, gt * 128:(gt + 1) * 128],
                        qT_sb[:, ht, :],
                        start=(ht == 0),
                        stop=(ht == ht_n - 1),
                    )
                nc.vector.tensor_copy(qp_sb[:, gt * bpc:(gt + 1) * bpc], qp_ps[:])

            # ---- bookkeeping tiles ----
            sums_sb = consts.tile([1, bpc * sc_n], F32)
            u_dram = dram.tile([bpc, s], F32)
            sums_dram = dram.tile([1, bpc], F32)


            # Delayed score-MM emission: the v-matmul for group (b,sc,gt)
            # is emitted after the *next* k_proj group so the PE never
            # stalls waiting for the tanh of the group it just produced.
            pending = []

            def flush_pending():
                while pending:
                    pending.pop(0)()

            acc_all = [None] * bpc
            for b in range(bpc):
                for sc in range(sc_n):
                    ssl = bass.ds(sc * sc_w, sc_w)
                    sc_ps = psum.tile([1, sc_w], F32, tag="sc", bufs=2)
                    for gt in range(gt_n):
                        kp_ps = psum.tile([128, sc_w], F32, tag="kp", bufs=3)
                        for ht in range(ht_n):
                            nc.tensor.matmul(
                                kp_ps[:],
                                wk_sb[:, ht, gt * 128:(gt + 1) * 128],
                                keys_sb[:, b * ht_n + ht, ssl],
                                start=(ht == 0),
                                stop=(ht == ht_n - 1),
                            )
                        comb = combp.tile([128, sc_w], kd, tag="comb")
                        nc.scalar.activation(
                            comb[:], kp_ps[:], AF.Tanh,
                            bias=qp_sb[:, gt * bpc + b: gt * bpc + b + 1],
                        )
                        if pending:
                            pending.pop(0)()

                        def score_mm(sc_ps=sc_ps, comb=comb, gt=gt, b=b, sc=sc, ssl=ssl):
                            nc.tensor.matmul(
                                sc_ps[:],
                                v_sb[:, gt:gt + 1],
                                comb[:],
                                start=(gt == 0),
                                stop=(gt == gt_n - 1),
                            )
                            if gt == gt_n - 1:
                                chunk_epi(b=b, sc=sc, ssl=ssl, sc_ps=sc_ps)
                        pending.append(score_mm)

                    # chunk epilogue (depends on sc_ps accumulation group)
                    def chunk_epi(b, sc, ssl, sc_ps):
                        nc.vector.copy_predicated(
                            sc_ps[:],
                            mask_sb[0:1, b * s + sc * sc_w: b * s + (sc + 1) * sc_w],
                            neg_sb[:],
                        )
                        stage = work.tile([1, sc_w], F32, tag="stage", bufs=3)
                        nc.scalar.activation(
                            stage[:], sc_ps[:], AF.Exp,
                            accum_out=sums_sb[0:1, b * sc_n + sc: b * sc_n + sc + 1],
                        )
                        nc.sync.dma_start(u_dram[b:b + 1, ssl], stage[:])
                        urep = work.tile([128, sc_w], F32, tag="urep", bufs=3)
                        nc.gpsimd.dma_start(
                            urep[:], u_dram[b:b + 1, ssl].to_broadcast([128, sc_w])
                        )
                        if sc == 0:
                            acc_all[b] = work.tile(
                                [128, ht_n, sc_n], F32, tag="accall", bufs=2,
                                name="acc_all",
                            )
                        for ht in range(ht_n):
                            junk = work.tile([128, sc_w], F32, tag="junk", bufs=2)
                            nc.vector.scalar_tensor_tensor(
                                out=junk[:],
                                in0=keys_sb[:, b * ht_n + ht, ssl].bitcast(F32),
                                scalar=1.0,
                                in1=urep[:],
                                op0=ALU.mult,
                                op1=ALU.mult,
                                accum_out=acc_all[b][:, ht, sc:sc + 1],
                            )

                        if sc == sc_n - 1:
                            bsum = work.tile([1, 1], F32, tag="bsum", bufs=4)
                            nc.vector.reduce_sum(
                                bsum[:],
                                sums_sb[0:1, b * sc_n:(b + 1) * sc_n],
                                axis=AX.X,
                            )
                            nc.sync.dma_start(sums_dram[0:1, b:b + 1], bsum[:])
                            srep = work.tile([128, 1], F32, tag="srep", bufs=4)
                            nc.gpsimd.dma_start(
                                srep[:], sums_dram[0:1, b:b + 1].to_broadcast([128, 1])
                            )
                            rrep = work.tile([128, 1], F32, tag="rrep", bufs=4)
                            nc.vector.reciprocal(rrep[:], srep[:])
                            acc4 = work.tile([128, ht_n], F32, tag="acc4", bufs=4)
                            nc.vector.reduce_sum(acc4[:], acc_all[b][:], axis=AX.X)
                            ctxc = work.tile([128, ht_n], F32, tag="ctxc", bufs=4)
                            nc.vector.tensor_scalar_mul(ctxc[:], acc4[:], rrep[:])
                            dst = bass.AP(
                                tensor=ctx_d.ap().tensor,
                                offset=b * h,
                                ap=[[1, 128], [128, ht_n]],
                            )
                            nc.sync.dma_start(dst, ctxc[:])

            flush_pending()

            # ---- weights output: w = u / sum ----
            u_sb = consts.tile([bpc, s], F32)
            nc.sync.dma_start(u_sb[:], u_dram[:])
            sums_t = work.tile([bpc, 1], F32, tag="sumst", bufs=1)
            nc.sync.dma_start(sums_t[:], sums_dram.rearrange("o b -> b o"))
            rcp4 = work.tile([bpc, 1], F32, tag="rcp4", bufs=1)
            nc.vector.reciprocal(rcp4[:], sums_t[:])
            nc.vector.tensor_scalar_mul(u_sb[:], u_sb[:], rcp4[:])
            nc.sync.dma_start(w_d.ap(), u_sb[:])

    nc.compile()
    return nc


_NC_CACHE = {}


def get_program(**kw):
    key = tuple(sorted(kw.items()))
    if key not in _NC_CACHE:
        _NC_CACHE[key] = build_program(**kw)
    return _NC_CACHE[key]


def make_in_maps(query, keys, mask, Wq, Wk, v, n_cores=N_CORES):
    """Host-side sharding + layout-only transforms (no math)."""
    query = np.asarray(query, dtype=np.float32)
    keys = np.asarray(keys, dtype=np.float32)
    mask = np.asarray(mask)
    wkT = np.ascontiguousarray(np.asarray(Wk, np.float32).T)
    wqT = np.ascontiguousarray(np.asarray(Wq, np.float32).T)
    v4 = np.ascontiguousarray(np.asarray(v, np.float32).reshape(-1, 128).T)
    bpc = query.shape[0] // n_cores
    in_maps = []
    for c in range(n_cores):
        sl = slice(c * bpc, (c + 1) * bpc)
        in_maps.append({
            "keysT": np.ascontiguousarray(keys[sl].transpose(0, 2, 1)),
            "queryT": np.ascontiguousarray(query[sl].T),
            "mask_u8": np.ascontiguousarray(mask[sl]).reshape(1, -1).view(np.uint8),
            "wkT": wkT,
            "wqT": wqT,
            "v4": v4,
        })
    return in_maps


def kernel(query, keys, mask, Wq, Wk, v):
    nc = get_program()
    in_maps = make_in_maps(query, keys, mask, Wq, Wk, v)
    res = bass_utils.run_bass_kernel_spmd(nc, in_maps, core_ids=list(range(N_CORES)))
    context = np.concatenate([res.results[c]["ctx"] for c in range(N_CORES)], axis=0)
    weights = np.concatenate([res.results[c]["w"] for c in range(N_CORES)], axis=0)
    return context, weights


# revision 10
# speedup vs baseline: 1.4008x; 1.4008x over previous
"""AdditiveAttention Trainium2 kernel.

reference:
    q_proj  = query @ Wq.T                              (B,H)
    k_proj  = einsum('bsh,gh->bsg', keys, Wk)           (B,S,H)
    scores  = einsum('bsh,h->bs', tanh(q_proj[:,None,:]+k_proj), v)
    scores  = where(mask, -1e9, scores)
    weights = softmax(scores, axis=1)                   (B,S)
    context = einsum('bs,bsh->bh', weights, keys)       (B,H)
    return (context, weights)

Sharding: data-parallel over batch, 4 batches per core on 8 cores.
Wk/Wq/v replicated. Host does layout-only transforms (transposes /
reshapes); all FLOPs run on device.

Per-core device pipeline (B'=4 local batches):
  - keysT (h on partitions) resident in SBUF: (128, 16, 2048) f32.
  - k_projT tile (g=128, s=512) = sum_ht WkT[ht,g].T @ keysT[ht,s]
    as fp32r matmuls (full PE rate at N=512), accumulated in PSUM.
  - tanh fused with q_proj bias on ScalarE: tanh(kp + qp[g]) -> SBUF.
  - scores chunk (1,512) = sum_gt v[gt].T @ combined[gt]  (M=1 matmuls).
  - mask (copy_predicated to -88) + Exp (accum_out = chunk sum) on chunk.
  - u chunk -> DRAM bounce -> DMA partition-broadcast (128,512) ->
    context via fused tensor_tensor_reduce on VectorE against keysT
    (contraction over s without needing a second keys layout).
  - weights = u * 1/sum  (per-batch sums), context scaled likewise.
"""

import sys

for _p in ("/opt/trn_rl_repo",):
    if _p not in sys.path:
        sys.path.insert(0, _p)

import numpy as np

import concourse.bass as bass
import concourse.bacc as bacc
import concourse.tile as tile
from concourse import mybir
from concourse import bass_utils

F32 = mybir.dt.float32
F32R = mybir.dt.float32r
U8 = mybir.dt.uint8
AF = mybir.ActivationFunctionType
ALU = mybir.AluOpType
AX = mybir.AxisListType

B, S, H = 32, 2048, 512
N_CORES = 8
BPC = B // N_CORES  # batches per core

NEG_MASK = -88.0  # exp(-88) == 0 in fp32; avoids feeding -1e9 to the Exp LUT


def build_program(bpc=BPC, s=S, h=H, kproj_dtype=F32R):
    """Build + compile the per-core Bass program (SPMD, same on all cores)."""
    nc = bacc.Bacc("TRN2", target_bir_lowering=False, debug=False)
    ht_n = h // 128   # tiles along contraction h
    gt_n = h // 128   # tiles along output g
    sc_n = s // 512   # 512-wide score chunks
    sc_w = 512

    kd = kproj_dtype
    keysT_d = nc.dram_tensor("keysT", (bpc, h, s), kd, kind="ExternalInput")
    qT_d = nc.dram_tensor("queryT", (h, bpc), F32, kind="ExternalInput")
    mask_d = nc.dram_tensor("mask_u8", (1, bpc * s), U8, kind="ExternalInput")
    wkT_d = nc.dram_tensor("wkT", (h, h), kd, kind="ExternalInput")
    wqT_d = nc.dram_tensor("wqT", (h, h), F32, kind="ExternalInput")
    v_d = nc.dram_tensor("v4", (128, gt_n), kd, kind="ExternalInput")
    ctx_d = nc.dram_tensor("ctx", (bpc, h), F32, kind="ExternalOutput")
    w_d = nc.dram_tensor("w", (bpc, s), F32, kind="ExternalOutput")

    with tile.TileContext(nc) as tc:
        with (
            tc.tile_pool(name="consts", bufs=1) as consts,
            tc.tile_pool(name="keysp", bufs=1) as keysp,
            tc.tile_pool(name="work", bufs=3) as work,
            tc.tile_pool(name="combp", bufs=4) as combp,
            tc.tile_pool(name="psum", bufs=3, space="PSUM") as psum,
            tc.tile_pool(name="dram", bufs=1, space="DRAM") as dram,
        ):
            # ---- resident inputs ----
            wk_sb = consts.tile([128, ht_n, h], kd)
            nc.sync.dma_start(wk_sb[:], wkT_d.ap().rearrange("(ht p) g -> p ht g", p=128))
            wq_sb = consts.tile([128, ht_n, h], F32)
            nc.sync.dma_start(wq_sb[:], wqT_d.ap().rearrange("(ht p) g -> p ht g", p=128))
            qT_sb = consts.tile([128, ht_n, bpc], F32)
            nc.sync.dma_start(qT_sb[:], qT_d.ap().rearrange("(ht p) c -> p ht c", p=128))
            v_sb = consts.tile([128, gt_n], kd)
            nc.sync.dma_start(v_sb[:], v_d.ap())
            mask_sb = consts.tile([1, bpc * s], U8)
            nc.sync.dma_start(mask_sb[:], mask_d.ap())
            neg_sb = consts.tile([1, sc_w], F32)
            nc.vector.memset(neg_sb[:], NEG_MASK)

            # keys after the small consts so they don't block the q_proj phase
            keys_sb = keysp.tile([128, bpc * ht_n, s], kd)
            keysT_r = keysT_d.ap().rearrange("b (ht p) s -> p (b ht) s", p=128)
            for b in range(bpc):
                for ht in range(ht_n):
                    i = b * ht_n + ht
                    nc.sync.dma_start(keys_sb[:, i, :], keysT_r[:, i, :])

            # ---- q_proj (exact fp32; tiny) ----
            qp_sb = consts.tile([128, gt_n * bpc], F32)
            for gt in range(gt_n):
                qp_ps = psum.tile([128, bpc], F32, tag="kp", bufs=4)
                for ht in range(ht_n):
                    nc.tensor.matmul(
                        qp_ps[:],
                        wq_sb[:, ht, gt * 128:(gt + 1) * 128],
                        qT_sb[:, ht, :],
                        start=(ht == 0),
                        stop=(ht == ht_n - 1),
                    )
                nc.vector.tensor_copy(qp_sb[:, gt * bpc:(gt + 1) * bpc], qp_ps[:])

            # ---- bookkeeping tiles ----
            sums_sb = consts.tile([1, bpc * sc_n], F32)
            u_dram = dram.tile([bpc, s], F32)
            sums_dram = dram.tile([1, bpc], F32)


            # Delayed score-MM emission: the v-matmul for (b,sc,gt) is
            # emitted after the *next* k_proj group so the PE never stalls
            # waiting on the tanh of the group it just produced.
            #
            # Loop order per (b, gt): hold one WkT weight tile (lhsT) and
            # stream all sc chunks through it before switching, so
            # consecutive matmuls share LDWEIGHTS.
            pending = []

            def flush_pending():
                while pending:
                    pending.pop(0)()

            def chunk_epi(b, sc, ssl, sc_ps):
                # masking + exp on the finished (1,512) score chunk
                nc.vector.copy_predicated(
                    sc_ps[:],
                    mask_sb[0:1, b * s + sc * sc_w: b * s + (sc + 1) * sc_w],
                    neg_sb[:],
                )
                stage = work.tile([1, sc_w], F32, tag="stage", bufs=3)
                nc.scalar.activation(
                    stage[:], sc_ps[:], AF.Exp,
                    accum_out=sums_sb[0:1, b * sc_n + sc: b * sc_n + sc + 1],
                )
                nc.sync.dma_start(u_dram[b:b + 1, ssl], stage[:])

                if sc == sc_n - 1:
                    # batch b complete: sums, 1/sum, context over full rows
                    bsum = work.tile([1, 1], F32, tag="bsum", bufs=4)
                    nc.vector.reduce_sum(
                        bsum[:], sums_sb[0:1, b * sc_n:(b + 1) * sc_n], axis=AX.X
                    )
                    nc.sync.dma_start(sums_dram[0:1, b:b + 1], bsum[:])
                    srep = work.tile([128, 1], F32, tag="srep", bufs=4)
                    nc.sync.dma_start(
                        srep[:], sums_dram[0:1, b:b + 1].to_broadcast([128, 1])
                    )
                    rrep = work.tile([128, 1], F32, tag="rrep", bufs=4)
                    nc.vector.reciprocal(rrep[:], srep[:])
                    urep = work.tile([128, s], F32, tag="urep", bufs=2)
                    nc.sync.dma_start(
                        urep[:], u_dram[b:b + 1, :].to_broadcast([128, s])
                    )
                    acc4 = work.tile([128, ht_n], F32, tag="acc4", bufs=4)
                    for ht in range(ht_n):
                        junk = work.tile([128, s], F32, tag="junk", bufs=1)
                        nc.vector.scalar_tensor_tensor(
                            out=junk[:],
                            in0=keys_sb[:, b * ht_n + ht, :].bitcast(F32),
                            scalar=1.0,
                            in1=urep[:],
                            op0=ALU.mult,
                            op1=ALU.mult,
                            accum_out=acc4[:, ht:ht + 1],
                        )
                    ctxc = work.tile([128, ht_n], F32, tag="ctxc", bufs=4)
                    nc.vector.tensor_scalar_mul(ctxc[:], acc4[:], rrep[:])
                    dst = bass.AP(
                        tensor=ctx_d.ap().tensor,
                        offset=b * h,
                        ap=[[1, 128], [128, ht_n]],
                    )
                    nc.sync.dma_start(dst, ctxc[:])

            for b in range(bpc):
                sc_pss = [
                    psum.tile([1, sc_w], F32, tag="sc", bufs=4, name="sc_ps")
                    for _ in range(sc_n)
                ]
                for gt in range(gt_n):
                    kp_pss = [
                        psum.tile([128, sc_w], F32, tag="kp", bufs=4, name="kp_ps")
                        for _ in range(sc_n)
                    ]
                    for ht in range(ht_n):
                        for sc in range(sc_n):
                            nc.tensor.matmul(
                                kp_pss[sc][:],
                                wk_sb[:, ht, gt * 128:(gt + 1) * 128],
                                keys_sb[:, b * ht_n + ht,
                                        bass.ds(sc * sc_w, sc_w)],
                                start=(ht == 0),
                                stop=(ht == ht_n - 1),
                            )
                    for sc in range(sc_n):
                        comb = combp.tile([128, sc_w], kd, tag="comb")
                        nc.scalar.activation(
                            comb[:], kp_pss[sc][:], AF.Tanh,
                            bias=qp_sb[:, gt * bpc + b: gt * bpc + b + 1],
                        )
                        if pending:
                            pending.pop(0)()

                        def score_mm(sc_ps=sc_pss[sc], comb=comb, gt=gt, b=b,
                                     sc=sc):
                            nc.tensor.matmul(
                                sc_ps[:],
                                v_sb[:, gt:gt + 1],
                                comb[:],
                                start=(gt == 0),
                                stop=(gt == gt_n - 1),
                            )
                            if gt == gt_n - 1:
                                chunk_epi(b=b, sc=sc,
                                          ssl=bass.ds(sc * sc_w, sc_w),
                                          sc_ps=sc_ps)
                        pending.append(score_mm)

            flush_pending()

            # ---- weights output: w = u / sum ----
            u_sb = work.tile([bpc, s], F32, tag="urep", bufs=2, name="u_sb")
            nc.sync.dma_start(u_sb[:], u_dram[:])
            sums_t = work.tile([bpc, 1], F32, tag="sumst", bufs=1)
            nc.sync.dma_start(sums_t[:], sums_dram.rearrange("o b -> b o"))
            rcp4 = work.tile([bpc, 1], F32, tag="rcp4", bufs=1)
            nc.vector.reciprocal(rcp4[:], sums_t[:])
            nc.vector.tensor_scalar_mul(u_sb[:], u_sb[:], rcp4[:])
            nc.sync.dma_start(w_d.ap(), u_sb[:])

    nc.compile()
    return nc


_NC_CACHE = {}


def get_program(**kw):
    key = tuple(sorted(kw.items()))
    if key not in _NC_CACHE:
        _NC_CACHE[key] = build_program(**kw)
    return _NC_CACHE[key]


def make_in_maps(query, keys, mask, Wq, Wk, v, n_cores=N_CORES):
    """Host-side sharding + layout-only transforms (no math)."""
    query = np.asarray(query, dtype=np.float32)
    keys = np.asarray(keys, dtype=np.float32)
    mask = np.asarray(mask)
    wkT = np.ascontiguousarray(np.asarray(Wk, np.float32).T)
    wqT = np.ascontiguousarray(np.asarray(Wq, np.float32).T)
    v4 = np.ascontiguousarray(np.asarray(v, np.float32).reshape(-1, 128).T)
    bpc = query.shape[0] // n_cores
    in_maps = []
    for c in range(n_cores):
        sl = slice(c * bpc, (c + 1) * bpc)
        in_maps.append({
            "keysT": np.ascontiguousarray(keys[sl].transpose(0, 2, 1)),
            "queryT": np.ascontiguousarray(query[sl].T),
            "mask_u8": np.ascontiguousarray(mask[sl]).reshape(1, -1).view(np.uint8),
            "wkT": wkT,
            "wqT": wqT,
            "v4": v4,
        })
    return in_maps


def kernel(query, keys, mask, Wq, Wk, v):
    nc = get_program()
    in_maps = make_in_maps(query, keys, mask, Wq, Wk, v)
    res = bass_utils.run_bass_kernel_spmd(nc, in_maps, core_ids=list(range(N_CORES)))
    context = np.concatenate([res.results[c]["ctx"] for c in range(N_CORES)], axis=0)
    weights = np.concatenate([res.results[c]["w"] for c in range(N_CORES)], axis=0)
    return context, weights
